# revision 83
# baseline (speedup 1.0000x reference)
"""CGC MoE routing kernel for Trainium2, 8-core data-parallel over batch.

Problem (per reference):
  B=4096, D_FULL=1024, D_T1=D_T2=512, experts: 4 shared (on x_full),
  4 task-1 (on x_task1), 4 task-2 (on x_task2); each expert is a 2-layer
  ReLU MLP (hidden 512, out 256). Three softmax gates combine expert
  outputs into (out_sh, out1, out2), each [4096, 256] fp32.

Strategy: shard the batch 8 ways (512 rows/core), replicate weights.
Each core computes all 12 experts + gates for its shard; host concats.
All matmul operands are bf16 (host pre-casts), PSUM accumulates fp32.

Perf notes:
  - Everything in DRAM is bf16 (except small fp32 biases), halving HBM
    traffic vs fp32 staging; the 8 cores contend for shared chip HBM.
  - DMAs are batched into few large transfers (>=2KB per partition
    line) and issued on the sync-engine HWDGE queue; gpsimd no longer
    issues input DMAs (casting DMAs kept it 95us busy in v1).
  - All weights are preloaded up front; only the first expert's w1 and
    x1 are split into per-128-slice DMAs so the first matmul can start
    early. A dense, near-critical-path-only early DMA schedule matters
    a lot: the PE clock p-state drops after stalls, so supply gaps cost
    roughly double their nominal length.
  - The 48 K=1 bias matmuls of v1 are gone: b2 is pre-broadcast on the
    host to [128, OUT] and added into PSUM by the vector engine before
    the relu activation (saves ~10us of PE time).

Hazards learned on HW (do not reintroduce):
  - A PSUM bank whose last PE write was a transpose, reused via a
    DVE-copy + start=False accumulation, reads corrupted.
  - start=False accumulation onto a virgin (never PE-written) PSUM bank
    acts as an implicit reset and wipes DVE-preloaded data.
  (This version sidesteps both: every accumulation group begins with
  start=True; the bias is TT-added into PSUM after the matmuls.)

Layout: activations kept as [feature, batch]:
  L1: H[h,b]  = sum_d W1[d,h].T @ X'[d,b]      (W1 tiles stationary)
  L2: EO[b,o] = sum_h H[h,b].T  @ W2[h,o]      (H tiles stationary)
      then PSUM += b2_bcast (vector), then ReLU -> eo bf16 (scalar).
  Gates: logits[g,b] = sum_d gW[d,g].T @ X'[d,b]; exp with per-partition
      bias on ACT; PE-transpose to [b,g]; columns pre-scaled by 1/rowsum
      so the combine uses normalized gates directly.
  Combine: acc[b,o] (+)= EO_e[b,o] * gate_col[b,1]
      (scalar_tensor_tensor fused multiply-add); acc DMAs straight out.
"""
import sys
import numpy as np
import ml_dtypes

sys.path.insert(0, "/opt/trn_rl_repo")

import concourse.bass as bass
import concourse.mybir as mybir
import concourse.tile as tile
import concourse.masks as masks
from concourse.bass_utils import run_bass_kernel_spmd

F32 = mybir.dt.float32
BF16 = mybir.dt.bfloat16
NPBF16 = ml_dtypes.bfloat16

B = 4096
N_CORES = 8
BC = B // N_CORES          # 512 rows per core
DF, D1, D2 = 1024, 512, 512
HID, OUT = 512, 256
NB = BC // 128             # 4 batch tiles per core
NH = HID // 128            # 4 hidden tiles
E = 4                      # experts per group
NG = (12, 8, 8)            # gate widths: gsh, g1, g2


def _legalize_waits(nc, max_waits: int = 1):
    """This walrus build supports a single sync wait per instruction;
    hoist extra waits onto standalone single-wait EventSemaphore
    instructions inserted just before (same engine, same order)."""
    uid = 0
    for f in nc.m.functions:
        for blk in f.blocks:
            out = []
            changed = False
            for inst in blk.instructions:
                si = inst.sync_info
                ow = list(si.on_wait) if si and si.on_wait else []
                if len(ow) > max_waits:
                    changed = True
                    for w in ow[:-max_waits]:
                        ev = mybir.InstEventSemaphore(
                            name=f"legalw-{uid}",
                            sync_info=mybir.SyncInfo(on_wait=[w], on_update=[]),
                        )
                        uid += 1
                        ev.engine = inst.engine
                        out.append(ev)
                    inst.sync_info = mybir.SyncInfo(
                        on_wait=ow[-max_waits:],
                        on_update=list(si.on_update) if si.on_update else [],
                    )
                out.append(inst)
            if changed:
                blk.instructions = out
    return nc


def _build_nc(legalize=True):
    nc = bass.Bass()

    def din(name, shape, dt=BF16):
        return nc.declare_dram_parameter(name, list(shape), dt, isOutput=False)

    # transposed x shards, packed [128, nd*BC]: [p, di*BC + b] = x[b, di*128+p]
    xfT = din("xfT", (128, (DF // 128) * BC))
    x1T = din("x1T", (128, (D1 // 128) * BC))
    x2T = din("x2T", (128, (D2 // 128) * BC))
    # host-packed expert weights: W1 [E, 128, nd*HID]; W2 per group
    # [128, E*NH*OUT]; b1 blob [128, 3*E*NH] fp32; b2 broadcast blob
    # [128, 3*E*OUT] bf16 (partition-replicated rows).
    t1W1 = din("t1W1", (E, 128, (D1 // 128) * HID))
    t2W1 = din("t2W1", (E, 128, (D2 // 128) * HID))
    shW1 = din("shW1", (E, 128, (DF // 128) * HID))
    t1W2 = din("t1W2", (128, E * NH * OUT))
    t2W2 = din("t2W2", (128, E * NH * OUT))
    shW2 = din("shW2", (128, E * NH * OUT))
    b1blob = din("b1blob", (128, 3 * E * NH), F32)
    b2blob = din("b2blob", (128, 3 * E * OUT))
    # gate weights blob [128, 8*12 + 4*8 + 4*8]; biases [96, 1] fp32
    # (each gate's bias starts at a 32-aligned partition: 0 / 32 / 64)
    gwblob = din("gwblob", (128, 8 * 12 + 4 * 8 + 4 * 8))
    gbblob = din("gbblob", (96, 1), F32)

    out_sh = nc.declare_dram_parameter("out_sh", [BC, OUT], F32, isOutput=True)
    out1 = nc.declare_dram_parameter("out1", [BC, OUT], F32, isOutput=True)
    out2 = nc.declare_dram_parameter("out2", [BC, OUT], F32, isOutput=True)

    with tile.TileContext(nc) as tc:
        _emit(nc, tc,
              {"xf": xfT, "x1": x1T, "x2": x2T},
              # expert groups in processing order: t1, sh, t2
              [("t1", t1W1, t1W2, D1 // 128),
               ("sh", shW1, shW2, DF // 128),
               ("t2", t2W1, t2W2, D2 // 128)],
              b1blob, b2blob, gwblob, gbblob,
              [out_sh, out1, out2])
    if legalize:
        _legalize_waits(nc)
    return nc


def _emit(nc, tc, xins, expert_groups, b1blob, b2blob, gwblob, gbblob, outs):
    from contextlib import ExitStack
    ctx = ExitStack()
    with ctx:
        xp = ctx.enter_context(tc.tile_pool(name="xp", bufs=1))
        wp = ctx.enter_context(tc.tile_pool(name="wp", bufs=1))
        bp = ctx.enter_context(tc.tile_pool(name="bp", bufs=1))
        hp = ctx.enter_context(tc.tile_pool(name="hp", bufs=2))
        eop = ctx.enter_context(tc.tile_pool(name="eop", bufs=12))
        eosp = ctx.enter_context(tc.tile_pool(name="eosp", bufs=4))
        gp = ctx.enter_context(tc.tile_pool(name="gp", bufs=1))
        accp = ctx.enter_context(tc.tile_pool(name="accp", bufs=1))
        misc = ctx.enter_context(tc.tile_pool(name="misc", bufs=1))
        # PSUM: 8 banks. ps1 gets 4 (L1 d-outer first expert + gate
        # logits share the tag), ps2 gets 4 (L2 + gate transposes).
        ps1 = ctx.enter_context(tc.tile_pool(name="ps1", bufs=4, space="PSUM"))
        ps2 = ctx.enter_context(tc.tile_pool(name="ps2", bufs=4, space="PSUM"))

        # ---- input DMAs -------------------------------------------------
        # x1 per-slice on gpsimd (parallel with sync's w1_t1e0 slices so
        # the first matmul starts early); everything else batched on
        # the sync HWDGE queue in consumption order.
        xt = {}
        x1 = xp.tile([128, (D1 // 128) * BC], BF16, tag="x_x1")
        for di in range(D1 // 128):
            nc.gpsimd.dma_start(x1[:, di * BC:(di + 1) * BC],
                                xins["x1"][:, di * BC:(di + 1) * BC])
        xt["t1"] = x1

        w1tiles = {}   # (group, e) -> tile
        g0, W1_0, _, nd0 = expert_groups[0]
        w0 = wp.tile([128, nd0 * HID], BF16, tag="w1_t1_0")
        for di in range(nd0):
            nc.sync.dma_start(w0[:, di * HID:(di + 1) * HID],
                              W1_0[0][:, di * HID:(di + 1) * HID])
        w1tiles[(g0, 0)] = w0

        b1t = bp.tile([128, 3 * E * NH], F32, tag="b1")
        nc.sync.dma_start(b1t[:], b1blob[:])
        # w2_t1's e0 slice rides ahead of b2/w1_e1-3 so L2_e0 doesn't
        # stall (~2.8us gap otherwise); the rest follows after w1_e3.
        w2tiles = {}
        w2_0 = wp.tile([128, E * NH * OUT], BF16, tag="w2_t1")
        w2tiles[g0] = w2_0
        EW2 = NH * OUT
        nc.sync.dma_start(w2_0[:, 0:EW2], expert_groups[0][2][:, 0:EW2])
        # w1_e1 rides ahead of the t1 b2 slice: a late bias only delays
        # a vector TT (single cost), a late w1 stalls the PE (double
        # cost via the p-state ramp).
        b2t = bp.tile([128, 3 * E * OUT], BF16, tag="b2")
        EB2 = E * OUT
        t = wp.tile([128, nd0 * HID], BF16, tag="w1_t1_1")
        nc.sync.dma_start(t[:], W1_0[1])
        w1tiles[(g0, 1)] = t
        nc.sync.dma_start(b2t[:, 0:EB2], b2blob[:, 0:EB2])

        # remaining t1 w1 + group w2
        for e in range(2, E):
            t = wp.tile([128, nd0 * HID], BF16, tag=f"w1_t1_{e}")
            nc.sync.dma_start(t[:], W1_0[e])
            w1tiles[(g0, e)] = t
        nc.sync.dma_start(w2_0[:, EW2:], expert_groups[0][2][:, EW2:])

        # xf, x2, gates
        xf = xp.tile([128, (DF // 128) * BC], BF16, tag="x_xf")
        nc.sync.dma_start(xf[:], xins["xf"][:])
        xt["sh"] = xf
        x2 = xp.tile([128, (D2 // 128) * BC], BF16, tag="x_x2")
        nc.sync.dma_start(x2[:], xins["x2"][:])
        xt["t2"] = x2
        gwt = gp.tile([128, 8 * 12 + 4 * 8 + 4 * 8], BF16, tag="gw")
        nc.sync.dma_start(gwt[:], gwblob[:])
        gbt = gp.tile([96, 1], F32, tag="gb")
        nc.sync.dma_start(gbt[:], gbblob[:])

        # sh and t2 weights (each group's b2 slice follows its first w1)
        for gi in (1, 2):
            group, W1, W2, nd = expert_groups[gi]
            for e in range(E):
                t = wp.tile([128, nd * HID], BF16, tag=f"w1_{group}_{e}")
                nc.sync.dma_start(t[:], W1[e])
                w1tiles[(group, e)] = t
                if e == 0:
                    nc.sync.dma_start(b2t[:, gi * EB2:(gi + 1) * EB2],
                                      b2blob[:, gi * EB2:(gi + 1) * EB2])
            t2w = wp.tile([128, E * NH * OUT], BF16, tag=f"w2_{group}")
            nc.sync.dma_start(t2w[:], W2[:])
            w2tiles[group] = t2w

        # identity for PE transpose (memset/iota invalid at bf16:
        # build fp32, cast-copy)
        ident32 = misc.tile([128, 128], F32, tag="ident32")
        masks.make_identity(nc, ident32[:])
        ident = misc.tile([128, 128], BF16, tag="ident")
        nc.vector.tensor_copy(ident[:], ident32[:])

        # accumulator tiles [128, OUT] per output per b-tile
        acc = [[accp.tile([128, OUT], F32, name=f"acc{o}_{bi}",
                          tag=f"acc{o}_{bi}")
                for bi in range(NB)] for o in range(3)]
        acc_init = [[False] * NB for _ in range(3)]

        gidx_of = {"t1": 0, "sh": 1, "t2": 2}

        # ---- expert bodies ---------------------------------------------
        def emit_expert_l1(group, e, nd, d_outer=False):
            x = xt[group]
            w1 = w1tiles[(group, e)]
            b1base = gidx_of[group] * E * NH + e * NH
            h = hp.tile([128, NH * BC], BF16, name="h", tag="h")
            if d_outer:
                ps = [ps1.tile([128, BC], F32, name=f"p1_{hi}", tag="p1")
                      for hi in range(NH)]
                for di in range(nd):
                    for hi in range(NH):
                        nc.tensor.matmul(
                            ps[hi][:],
                            w1[:, di * HID + hi * 128: di * HID + (hi + 1) * 128],
                            x[:, di * BC:(di + 1) * BC],
                            start=(di == 0), stop=(di == nd - 1))
                for hi in range(NH):
                    nc.scalar.activation(h[:, hi * BC:(hi + 1) * BC],
                                         ps[hi][:],
                                         mybir.ActivationFunctionType.Relu,
                                         bias=b1t[:, b1base + hi: b1base + hi + 1])
            else:
                for hi in range(NH):
                    p1 = ps1.tile([128, BC], F32, name="p1", tag="p1")
                    for di in range(nd):
                        nc.tensor.matmul(
                            p1[:],
                            w1[:, di * HID + hi * 128: di * HID + (hi + 1) * 128],
                            x[:, di * BC:(di + 1) * BC],
                            start=(di == 0), stop=(di == nd - 1))
                    nc.scalar.activation(h[:, hi * BC:(hi + 1) * BC], p1[:],
                                         mybir.ActivationFunctionType.Relu,
                                         bias=b1t[:, b1base + hi: b1base + hi + 1])
            return h

        def emit_expert_l2(group, e, h, gate_cols=None):
            w2 = w2tiles[group]
            b2base = (gidx_of[group] * E + e) * OUT
            eos = []
            p2s = []
            for bi in range(NB):
                p2 = ps2.tile([128, OUT], F32, name="p2", tag="p2")
                for hi in range(NH):
                    nc.tensor.matmul(
                        p2[:],
                        h[:, hi * BC + bi * 128: hi * BC + (bi + 1) * 128],
                        w2[:, (e * NH + hi) * OUT:(e * NH + hi + 1) * OUT],
                        start=(hi == 0), stop=(hi == NH - 1))
                # bias along the free dim: vector adds the pre-broadcast
                # b2 row into PSUM, then scalar does the ReLU.
                nc.vector.tensor_add(p2[:], p2[:],
                                     b2t[:, b2base:b2base + OUT])
                eo = eop.tile([128, OUT], BF16, name="eo", tag="eo")
                nc.scalar.activation(eo[:], p2[:],
                                     mybir.ActivationFunctionType.Relu)
                eos.append(eo)
                p2s.append(p2)
                if gate_cols is not None:
                    # final-expert only: emit this btile's combines right
                    # away so the drain's vector queue isn't program-
                    # ordered behind the later btiles' bias adds.
                    _combine_bt(group, e, bi, eo, p2, gate_cols)
            return eos, p2s

        # expert -> (output index, gate set, gate column) contributions
        # gates: gsh over [t1(0-3), t2(4-7), sh(8-11)]
        #        g1  over [t1(0-3), sh(4-7)]; g2 over [t2(0-3), sh(4-7)]
        def contributions(group, e):
            if group == "t1":
                return [(0, 0, e), (1, 1, e)]
            elif group == "t2":
                return [(0, 0, 4 + e), (2, 2, e)]
            else:
                return [(0, 0, 8 + e), (1, 1, 4 + e), (2, 2, 4 + e)]

        # processing order: t1(0..3), sh(0..3), t2(0..3)
        # out1 finishes at sh_3 (its DMA overlaps the t2 group);
        # out_sh/out2 finish at t2_3.
        def _is_last_contrib(group, e, o):
            if o == 1:
                return group == "sh" and e == E - 1
            return group == "t2" and e == E - 1

        def _combine_bt(group, e, bi, eo, p2, gate_cols):
            emit_combine(group, e, [eo] * NB, gate_cols,
                         [p2] * NB, only_bi=bi)

        def emit_combine(group, e, eos, gate_cols, p2s=None, only_bi=None):
            for bi in range(NB):
                if only_bi is not None and bi != only_bi:
                    continue
                eo = eos[bi]
                for (o, gs, col) in contributions(group, e):
                    g_ap = gate_cols[gs][bi][:, col:col + 1]
                    a = acc[o][bi]
                    if (o == 0 and group == "t2" and e == E - 1
                            and p2s is not None):
                        # Final-expert drain split: p2 already holds
                        # z + b2 (the TT add ran before the ReLU), and
                        # gates are softmax-positive, so
                        # relu(g*p2) == g*relu(z+b2). Scalar does the
                        # scaled relu, gpsimd the accumulate — keeping
                        # the tail off the serialized vector queue.
                        eo_s = eosp.tile([128, OUT], BF16, name="eo_s",
                                         tag="eo_s")
                        nc.scalar.activation(eo_s[:], p2s[bi][:],
                                             mybir.ActivationFunctionType.Relu,
                                             scale=g_ap)
                        nc.gpsimd.tensor_add(a[:], a[:], eo_s[:])
                    elif not acc_init[o][bi]:
                        nc.vector.tensor_scalar_mul(a[:], eo[:], g_ap)
                        acc_init[o][bi] = True
                    else:
                        nc.vector.scalar_tensor_tensor(
                            a[:], eo[:], g_ap, a[:],
                            op0=mybir.AluOpType.mult,
                            op1=mybir.AluOpType.add)
                    if _is_last_contrib(group, e, o):
                        nc.sync.dma_start(
                            outs[o][bi * 128:(bi + 1) * 128, :], a[:])

        # ---- gates ------------------------------------------------------
        def emit_gates():
            gate_cols = []
            gw_off = 0
            for gi in range(3):
                ng = NG[gi]
                x = {0: xt["sh"], 1: xt["t1"], 2: xt["t2"]}[gi]
                nd = {0: DF // 128, 1: D1 // 128, 2: D2 // 128}[gi]
                lg = ps1.tile([ng, BC], F32, name="lg", tag="p1")
                for di in range(nd):
                    nc.tensor.matmul(
                        lg[:], gwt[:, gw_off + di * ng: gw_off + (di + 1) * ng],
                        x[:, di * BC:(di + 1) * BC],
                        start=(di == 0), stop=(di == nd - 1))
                gw_off += nd * ng
                eg = gp.tile([ng, BC], BF16, name=f"eg{gi}", tag=f"eg{gi}")
                nc.scalar.activation(eg[:], lg[:],
                                     mybir.ActivationFunctionType.Exp,
                                     bias=gbt[gi * 32:gi * 32 + ng, :])
                cols = []
                for bi in range(NB):
                    pt = ps2.tile([128, ng], BF16, name="gtr", tag="p2")
                    nc.tensor.transpose(pt[:], eg[:, bi * 128:(bi + 1) * 128],
                                        ident[:ng, :ng])
                    ct = gp.tile([128, ng], F32, name=f"gc{gi}_{bi}",
                                 tag=f"gc{gi}_{bi}")
                    nc.vector.tensor_copy(ct[:], pt[:])
                    st = gp.tile([128, 1], F32, name=f"gs{gi}_{bi}",
                                 tag=f"gs{gi}_{bi}")
                    nc.vector.tensor_reduce(st[:], ct[:],
                                            axis=mybir.AxisListType.X,
                                            op=mybir.AluOpType.add)
                    rt = gp.tile([128, 1], F32, name=f"gr{gi}_{bi}",
                                 tag=f"gr{gi}_{bi}")
                    nc.vector.reciprocal(rt[:], st[:])
                    # pre-scale the gate columns: combine uses these directly
                    nc.vector.tensor_scalar_mul(ct[:], ct[:], rt[:])
                    cols.append(ct)
                gate_cols.append(cols)
            return gate_cols

        # ---- emission order --------------------------------------------
        # expert t1_0 (d-outer, earliest possible PE start), t1_1, t1_2,
        # then gates (xf has landed by then), then the deferred combines
        # and the remaining experts.
        h0 = emit_expert_l1("t1", 0, nd0, d_outer=True)
        eos_0, _ = emit_expert_l2("t1", 0, h0)
        h1 = emit_expert_l1("t1", 1, nd0)
        eos_1, _ = emit_expert_l2("t1", 1, h1)
        h2 = emit_expert_l1("t1", 2, nd0)
        eos_2, _ = emit_expert_l2("t1", 2, h2)
        gate_cols = emit_gates()
        emit_combine("t1", 0, eos_0, gate_cols)
        emit_combine("t1", 1, eos_1, gate_cols)
        emit_combine("t1", 2, eos_2, gate_cols)
        for (group, _, _, nd) in expert_groups:
            for e in range(E):
                if group == "t1" and e in (0, 1, 2):
                    continue
                inline = (group == "t2" and e == E - 1)
                h = emit_expert_l1(group, e, nd)
                eos, p2s = emit_expert_l2(group, e, h,
                                          gate_cols if inline else None)
                if not inline:
                    emit_combine(group, e, eos, gate_cols, p2s)


_NC_CACHE = None


def _pack_xT(x_rows):
    """[BC, D] -> [128, (D/128)*BC] bf16: [p, di*BC + b] = x[b, di*128+p]."""
    bc, d = x_rows.shape
    nd = d // 128
    return np.ascontiguousarray(
        x_rows.T.reshape(nd, 128, bc).transpose(1, 0, 2).reshape(128, nd * bc)
    ).astype(NPBF16)


def _pack_inputs(inputs):
    """Host-side packing into SBUF partition layouts (pure relayout)."""
    def pack_w1(w):           # [E, D, HID] -> [E, 128, (D/128)*HID] bf16
        e, dd, nn = w.shape
        nd = dd // 128
        return np.ascontiguousarray(
            w.reshape(e, nd, 128, nn).transpose(0, 2, 1, 3).reshape(e, 128, nd * nn)
        ).astype(NPBF16)

    def pack_w2(w):           # [E, HID, OUT] -> [128, E*NH*OUT] bf16
        e, hh, nn = w.shape
        nh = hh // 128
        return np.ascontiguousarray(
            w.reshape(e, nh, 128, nn).transpose(2, 0, 1, 3).reshape(128, e * nh * nn)
        ).astype(NPBF16)

    def pack_gw(w):           # [D, ng] -> [128, (D/128)*ng]
        dd, ng = w.shape
        nd = dd // 128
        return np.ascontiguousarray(
            w.reshape(nd, 128, ng).transpose(1, 0, 2).reshape(128, nd * ng))

    # b1 blob [128, 3*E*NH] fp32: [p, (gi*E+e)*NH + hi] = b1[gi][e, hi*128+p]
    b1 = np.stack([np.asarray(inputs[f"{g}_b1"], np.float32)
                   .reshape(E, NH, 128).transpose(2, 0, 1)
                   for g in ("t1", "sh", "t2")], axis=1)   # [128, 3, E, NH]
    b1blob = np.ascontiguousarray(b1.reshape(128, 3 * E * NH), np.float32)

    # b2 blob [128, 3*E*OUT] bf16, partition-replicated rows
    b2 = np.concatenate([np.asarray(inputs[f"{g}_b2"], np.float32).reshape(E * OUT)
                         for g in ("t1", "sh", "t2")])     # [3*E*OUT]
    b2blob = np.ascontiguousarray(
        np.broadcast_to(b2[None, :], (128, 3 * E * OUT))).astype(NPBF16)

    gwblob = np.ascontiguousarray(np.concatenate(
        [pack_gw(np.asarray(inputs["gsh_W"], np.float32)),
         pack_gw(np.asarray(inputs["g1_W"], np.float32)),
         pack_gw(np.asarray(inputs["g2_W"], np.float32))], axis=1)).astype(NPBF16)
    gbblob = np.zeros((96, 1), np.float32)
    gbblob[0:12, 0] = np.asarray(inputs["gsh_b"], np.float32)
    gbblob[32:40, 0] = np.asarray(inputs["g1_b"], np.float32)
    gbblob[64:72, 0] = np.asarray(inputs["g2_b"], np.float32)

    return {
        "t1W1": pack_w1(inputs["t1_W1"]), "t2W1": pack_w1(inputs["t2_W1"]),
        "shW1": pack_w1(inputs["sh_W1"]),
        "t1W2": pack_w2(inputs["t1_W2"]), "t2W2": pack_w2(inputs["t2_W2"]),
        "shW2": pack_w2(inputs["sh_W2"]),
        "b1blob": b1blob, "b2blob": b2blob,
        "gwblob": gwblob, "gbblob": gbblob,
    }


def kernel(**inputs):
    global _NC_CACHE
    if _NC_CACHE is None:
        _NC_CACHE = _build_nc()
    nc = _NC_CACHE

    shared = _pack_inputs(inputs)
    xf, x1, x2 = inputs["x_full"], inputs["x_task1"], inputs["x_task2"]

    in_maps = []
    for c in range(N_CORES):
        rows = slice(c * BC, (c + 1) * BC)
        m = dict(shared)
        m["xfT"] = _pack_xT(xf[rows])
        m["x1T"] = _pack_xT(x1[rows])
        m["x2T"] = _pack_xT(x2[rows])
        in_maps.append(m)

    res = run_bass_kernel_spmd(nc, in_maps, list(range(N_CORES)))
    out_sh = np.concatenate([res.results[c]["out_sh"] for c in range(N_CORES)])
    out1 = np.concatenate([res.results[c]["out1"] for c in range(N_CORES)])
    out2 = np.concatenate([res.results[c]["out2"] for c in range(N_CORES)])
    return (out_sh, out1, out2)


# revision 84
# speedup vs baseline: 1.1915x; 1.1915x over previous
"""CGC MoE routing kernel for Trainium2, 8-core data-parallel over batch.

Problem (per reference):
  B=4096, D_FULL=1024, D_T1=D_T2=512, experts: 4 shared (on x_full),
  4 task-1 (on x_task1), 4 task-2 (on x_task2); each expert is a 2-layer
  ReLU MLP (hidden 512, out 256). Three softmax gates combine expert
  outputs into (out_sh, out1, out2), each [4096, 256] fp32.

Strategy: shard the batch 8 ways (512 rows/core), replicate weights.
Each core computes all 12 experts + gates for its shard; host concats.
All matmul operands are bf16 (host pre-casts), PSUM accumulates fp32.

Perf notes:
  - Everything in DRAM is bf16 (except small fp32 biases), halving HBM
    traffic vs fp32 staging; the 8 cores contend for shared chip HBM.
  - DMAs are batched into few large transfers (>=2KB per partition
    line) and issued on the sync-engine HWDGE queue; gpsimd no longer
    issues input DMAs (casting DMAs kept it 95us busy in v1).
  - All weights are preloaded up front; only the first expert's w1 and
    x1 are split into per-128-slice DMAs so the first matmul can start
    early. A dense, near-critical-path-only early DMA schedule matters
    a lot: the PE clock p-state drops after stalls, so supply gaps cost
    roughly double their nominal length.
  - The 48 K=1 bias matmuls of v1 are gone: b2 is pre-broadcast on the
    host to [128, OUT] and added into PSUM by the vector engine before
    the relu activation (saves ~10us of PE time).

Hazards learned on HW (do not reintroduce):
  - A PSUM bank whose last PE write was a transpose, reused via a
    DVE-copy + start=False accumulation, reads corrupted.
  - start=False accumulation onto a virgin (never PE-written) PSUM bank
    acts as an implicit reset and wipes DVE-preloaded data.
  (This version sidesteps both: every accumulation group begins with
  start=True; the bias is TT-added into PSUM after the matmuls.)

Layout: activations kept as [feature, batch]:
  L1: H[h,b]  = sum_d W1[d,h].T @ X'[d,b]      (W1 tiles stationary)
  L2: EO[b,o] = sum_h H[h,b].T  @ W2[h,o]      (H tiles stationary)
      then PSUM += b2_bcast (vector), then ReLU -> eo bf16 (scalar).
  Gates: logits[g,b] = sum_d gW[d,g].T @ X'[d,b]; exp with per-partition
      bias on ACT; PE-transpose to [b,g]; columns pre-scaled by 1/rowsum
      so the combine uses normalized gates directly.
  Combine: acc[b,o] (+)= EO_e[b,o] * gate_col[b,1]
      (scalar_tensor_tensor fused multiply-add); acc DMAs straight out.
"""
import sys
import numpy as np
import ml_dtypes

sys.path.insert(0, "/opt/trn_rl_repo")

import concourse.bass as bass
import concourse.mybir as mybir
import concourse.tile as tile
import concourse.masks as masks
from concourse.bass_utils import run_bass_kernel_spmd

F32 = mybir.dt.float32
BF16 = mybir.dt.bfloat16
NPBF16 = ml_dtypes.bfloat16

B = 4096
N_CORES = 8
BC = B // N_CORES          # 512 rows per core
DF, D1, D2 = 1024, 512, 512
HID, OUT = 512, 256
NB = BC // 128             # 4 batch tiles per core
NH = HID // 128            # 4 hidden tiles
E = 4                      # experts per group
NG = (12, 8, 8)            # gate widths: gsh, g1, g2


def _legalize_waits(nc, max_waits: int = 1):
    """This walrus build supports a single sync wait per instruction;
    hoist extra waits onto standalone single-wait EventSemaphore
    instructions inserted just before (same engine, same order)."""
    uid = 0
    for f in nc.m.functions:
        for blk in f.blocks:
            out = []
            changed = False
            for inst in blk.instructions:
                si = inst.sync_info
                ow = list(si.on_wait) if si and si.on_wait else []
                if len(ow) > max_waits:
                    changed = True
                    for w in ow[:-max_waits]:
                        ev = mybir.InstEventSemaphore(
                            name=f"legalw-{uid}",
                            sync_info=mybir.SyncInfo(on_wait=[w], on_update=[]),
                        )
                        uid += 1
                        ev.engine = inst.engine
                        out.append(ev)
                    inst.sync_info = mybir.SyncInfo(
                        on_wait=ow[-max_waits:],
                        on_update=list(si.on_update) if si.on_update else [],
                    )
                out.append(inst)
            if changed:
                blk.instructions = out
    return nc


def _build_nc(legalize=True):
    nc = bass.Bass()

    def din(name, shape, dt=BF16):
        return nc.declare_dram_parameter(name, list(shape), dt, isOutput=False)

    # transposed x shards, packed [128, nd*BC]: [p, di*BC + b] = x[b, di*128+p]
    xfT = din("xfT", (128, (DF // 128) * BC))
    x1T = din("x1T", (128, (D1 // 128) * BC))
    x2T = din("x2T", (128, (D2 // 128) * BC))
    # host-packed expert weights: W1 [E, 128, nd*HID]; W2 per group
    # [128, E*NH*OUT]; b1 blob [128, 3*E*NH] fp32; b2 broadcast blob
    # [128, 3*E*OUT] bf16 (partition-replicated rows).
    t1W1 = din("t1W1", (E, 128, (D1 // 128) * HID))
    t2W1 = din("t2W1", (E, 128, (D2 // 128) * HID))
    shW1 = din("shW1", (E, 128, (DF // 128) * HID))
    t1W2 = din("t1W2", (128, E * NH * OUT))
    t2W2 = din("t2W2", (128, E * NH * OUT))
    shW2 = din("shW2", (128, E * NH * OUT))
    b1blob = din("b1blob", (128, 3 * E * NH), F32)
    b2blob = din("b2blob", (128, 3 * E * OUT))
    # gate weights blob [128, 8*12 + 4*8 + 4*8]; biases [96, 1] fp32
    # (each gate's bias starts at a 32-aligned partition: 0 / 32 / 64)
    gwblob = din("gwblob", (128, 8 * 12 + 4 * 8 + 4 * 8))
    gbblob = din("gbblob", (96, 1), F32)

    out_sh = nc.declare_dram_parameter("out_sh", [BC, OUT], F32, isOutput=True)
    out1 = nc.declare_dram_parameter("out1", [BC, OUT], F32, isOutput=True)
    out2 = nc.declare_dram_parameter("out2", [BC, OUT], F32, isOutput=True)

    with tile.TileContext(nc) as tc:
        _emit(nc, tc,
              {"xf": xfT, "x1": x1T, "x2": x2T},
              # expert groups in processing order: t1, sh, t2
              [("t1", t1W1, t1W2, D1 // 128),
               ("sh", shW1, shW2, DF // 128),
               ("t2", t2W1, t2W2, D2 // 128)],
              b1blob, b2blob, gwblob, gbblob,
              [out_sh, out1, out2])
    if legalize:
        _legalize_waits(nc)
    return nc


def _emit(nc, tc, xins, expert_groups, b1blob, b2blob, gwblob, gbblob, outs):
    from contextlib import ExitStack
    ctx = ExitStack()
    with ctx:
        xp = ctx.enter_context(tc.tile_pool(name="xp", bufs=1))
        wp = ctx.enter_context(tc.tile_pool(name="wp", bufs=1))
        bp = ctx.enter_context(tc.tile_pool(name="bp", bufs=1))
        hp = ctx.enter_context(tc.tile_pool(name="hp", bufs=2))
        eop = ctx.enter_context(tc.tile_pool(name="eop", bufs=12))
        eosp = ctx.enter_context(tc.tile_pool(name="eosp", bufs=4))
        gp = ctx.enter_context(tc.tile_pool(name="gp", bufs=1))
        accp = ctx.enter_context(tc.tile_pool(name="accp", bufs=1))
        misc = ctx.enter_context(tc.tile_pool(name="misc", bufs=1))
        # PSUM: 8 banks. ps1 gets 4 (L1 d-outer first expert + gate
        # logits share the tag), ps2 gets 4 (L2 + gate transposes).
        ps1 = ctx.enter_context(tc.tile_pool(name="ps1", bufs=4, space="PSUM"))
        ps2 = ctx.enter_context(tc.tile_pool(name="ps2", bufs=4, space="PSUM"))

        # ---- input DMAs -------------------------------------------------
        # x1 per-slice on gpsimd (parallel with sync's w1_t1e0 slices so
        # the first matmul starts early); everything else batched on
        # the sync HWDGE queue in consumption order.
        xt = {}
        x1 = xp.tile([128, (D1 // 128) * BC], BF16, tag="x_x1")
        for di in range(D1 // 128):
            nc.gpsimd.dma_start(x1[:, di * BC:(di + 1) * BC],
                                xins["x1"][:, di * BC:(di + 1) * BC])
        xt["t1"] = x1

        w1tiles = {}   # (group, e) -> tile
        g0, W1_0, _, nd0 = expert_groups[0]
        w0 = wp.tile([128, nd0 * HID], BF16, tag="w1_t1_0")
        for di in range(nd0):
            nc.sync.dma_start(w0[:, di * HID:(di + 1) * HID],
                              W1_0[0][:, di * HID:(di + 1) * HID])
        w1tiles[(g0, 0)] = w0

        b1t = bp.tile([128, 3 * E * NH], F32, tag="b1")
        nc.sync.dma_start(b1t[:], b1blob[:])
        # w2_t1's e0 slice rides ahead of b2/w1_e1-3 so L2_e0 doesn't
        # stall (~2.8us gap otherwise); the rest follows after w1_e3.
        w2tiles = {}
        w2_0 = wp.tile([128, E * NH * OUT], BF16, tag="w2_t1")
        w2tiles[g0] = w2_0
        EW2 = NH * OUT
        nc.sync.dma_start(w2_0[:, 0:EW2], expert_groups[0][2][:, 0:EW2])
        # w1_e1 rides ahead of the t1 b2 slice: a late bias only delays
        # a vector TT (single cost), a late w1 stalls the PE (double
        # cost via the p-state ramp).
        b2t = bp.tile([128, 3 * E * OUT], BF16, tag="b2")
        EB2 = E * OUT
        t = wp.tile([128, nd0 * HID], BF16, tag="w1_t1_1")
        nc.sync.dma_start(t[:], W1_0[1])
        w1tiles[(g0, 1)] = t
        nc.sync.dma_start(b2t[:, 0:EB2], b2blob[:, 0:EB2])

        # remaining t1 w1 + group w2
        for e in range(2, E):
            t = wp.tile([128, nd0 * HID], BF16, tag=f"w1_t1_{e}")
            nc.sync.dma_start(t[:], W1_0[e])
            w1tiles[(g0, e)] = t
        nc.sync.dma_start(w2_0[:, EW2:], expert_groups[0][2][:, EW2:])

        # xf, x2, gates
        xf = xp.tile([128, (DF // 128) * BC], BF16, tag="x_xf")
        nc.sync.dma_start(xf[:], xins["xf"][:])
        xt["sh"] = xf
        x2 = xp.tile([128, (D2 // 128) * BC], BF16, tag="x_x2")
        nc.sync.dma_start(x2[:], xins["x2"][:])
        xt["t2"] = x2
        gwt = gp.tile([128, 8 * 12 + 4 * 8 + 4 * 8], BF16, tag="gw")
        nc.sync.dma_start(gwt[:], gwblob[:])
        gbt = gp.tile([96, 1], F32, tag="gb")
        nc.sync.dma_start(gbt[:], gbblob[:])

        # sh and t2 weights (each group's b2 slice follows its first w1)
        for gi in (1, 2):
            group, W1, W2, nd = expert_groups[gi]
            for e in range(E):
                t = wp.tile([128, nd * HID], BF16, tag=f"w1_{group}_{e}")
                nc.sync.dma_start(t[:], W1[e])
                w1tiles[(group, e)] = t
                if e == 0:
                    nc.sync.dma_start(b2t[:, gi * EB2:(gi + 1) * EB2],
                                      b2blob[:, gi * EB2:(gi + 1) * EB2])
            t2w = wp.tile([128, E * NH * OUT], BF16, tag=f"w2_{group}")
            nc.sync.dma_start(t2w[:], W2[:])
            w2tiles[group] = t2w

        # identity for PE transpose (memset/iota invalid at bf16:
        # build fp32, cast-copy)
        ident32 = misc.tile([128, 128], F32, tag="ident32")
        masks.make_identity(nc, ident32[:])
        ident = misc.tile([128, 128], BF16, tag="ident")
        nc.vector.tensor_copy(ident[:], ident32[:])

        # accumulator tiles [128, OUT] per output per b-tile
        acc = [[accp.tile([128, OUT], F32, name=f"acc{o}_{bi}",
                          tag=f"acc{o}_{bi}")
                for bi in range(NB)] for o in range(3)]
        acc_init = [[False] * NB for _ in range(3)]

        gidx_of = {"t1": 0, "sh": 1, "t2": 2}

        # ---- expert bodies ---------------------------------------------
        def emit_expert_l1(group, e, nd, d_outer=False):
            x = xt[group]
            w1 = w1tiles[(group, e)]
            b1base = gidx_of[group] * E * NH + e * NH
            h = hp.tile([128, NH * BC], BF16, name="h", tag="h")
            if d_outer:
                ps = [ps1.tile([128, BC], F32, name=f"p1_{hi}", tag="p1")
                      for hi in range(NH)]
                for di in range(nd):
                    for hi in range(NH):
                        nc.tensor.matmul(
                            ps[hi][:],
                            w1[:, di * HID + hi * 128: di * HID + (hi + 1) * 128],
                            x[:, di * BC:(di + 1) * BC],
                            start=(di == 0), stop=(di == nd - 1))
                for hi in range(NH):
                    nc.scalar.activation(h[:, hi * BC:(hi + 1) * BC],
                                         ps[hi][:],
                                         mybir.ActivationFunctionType.Relu,
                                         bias=b1t[:, b1base + hi: b1base + hi + 1])
            else:
                for hi in range(NH):
                    p1 = ps1.tile([128, BC], F32, name="p1", tag="p1")
                    for di in range(nd):
                        nc.tensor.matmul(
                            p1[:],
                            w1[:, di * HID + hi * 128: di * HID + (hi + 1) * 128],
                            x[:, di * BC:(di + 1) * BC],
                            start=(di == 0), stop=(di == nd - 1))
                    nc.scalar.activation(h[:, hi * BC:(hi + 1) * BC], p1[:],
                                         mybir.ActivationFunctionType.Relu,
                                         bias=b1t[:, b1base + hi: b1base + hi + 1])
            return h

        def emit_expert_l2(group, e, h):
            w2 = w2tiles[group]
            b2base = (gidx_of[group] * E + e) * OUT
            eos = []
            p2s = []
            for bi in range(NB):
                p2 = ps2.tile([128, OUT], F32, name="p2", tag="p2")
                for hi in range(NH):
                    nc.tensor.matmul(
                        p2[:],
                        h[:, hi * BC + bi * 128: hi * BC + (bi + 1) * 128],
                        w2[:, (e * NH + hi) * OUT:(e * NH + hi + 1) * OUT],
                        start=(hi == 0), stop=(hi == NH - 1))
                # bias along the free dim: vector adds the pre-broadcast
                # b2 row into PSUM, then scalar does the ReLU.
                nc.vector.tensor_add(p2[:], p2[:],
                                     b2t[:, b2base:b2base + OUT])
                eo = eop.tile([128, OUT], BF16, name="eo", tag="eo")
                nc.scalar.activation(eo[:], p2[:],
                                     mybir.ActivationFunctionType.Relu)
                eos.append(eo)
                p2s.append(p2)
            return eos, p2s

        # expert -> (output index, gate set, gate column) contributions
        # gates: gsh over [t1(0-3), t2(4-7), sh(8-11)]
        #        g1  over [t1(0-3), sh(4-7)]; g2 over [t2(0-3), sh(4-7)]
        def contributions(group, e):
            if group == "t1":
                return [(0, 0, e), (1, 1, e)]
            elif group == "t2":
                return [(0, 0, 4 + e), (2, 2, e)]
            else:
                return [(0, 0, 8 + e), (1, 1, 4 + e), (2, 2, 4 + e)]

        # processing order: t1(0..3), sh(0..3), t2(0..3)
        # out1 finishes at sh_3 (its DMA overlaps the t2 group);
        # out_sh/out2 finish at t2_3.
        def _is_last_contrib(group, e, o):
            if o == 1:
                return group == "sh" and e == E - 1
            return group == "t2" and e == E - 1

        def emit_combine(group, e, eos, gate_cols, p2s=None):
            for bi in range(NB):
                eo = eos[bi]
                for (o, gs, col) in contributions(group, e):
                    g_ap = gate_cols[gs][bi][:, col:col + 1]
                    a = acc[o][bi]
                    if (o == 0 and group == "t2" and e == E - 1
                            and p2s is not None):
                        # Final-expert drain split: p2 already holds
                        # z + b2 (the TT add ran before the ReLU), and
                        # gates are softmax-positive, so
                        # relu(g*p2) == g*relu(z+b2). Scalar does the
                        # scaled relu, gpsimd the accumulate — keeping
                        # the tail off the serialized vector queue.
                        eo_s = eosp.tile([128, OUT], BF16, name="eo_s",
                                         tag="eo_s")
                        nc.scalar.activation(eo_s[:], p2s[bi][:],
                                             mybir.ActivationFunctionType.Relu,
                                             scale=g_ap)
                        nc.gpsimd.tensor_add(a[:], a[:], eo_s[:])
                    elif not acc_init[o][bi]:
                        nc.vector.tensor_scalar_mul(a[:], eo[:], g_ap)
                        acc_init[o][bi] = True
                    else:
                        nc.vector.scalar_tensor_tensor(
                            a[:], eo[:], g_ap, a[:],
                            op0=mybir.AluOpType.mult,
                            op1=mybir.AluOpType.add)
                    if _is_last_contrib(group, e, o):
                        nc.sync.dma_start(
                            outs[o][bi * 128:(bi + 1) * 128, :], a[:])

        # ---- gates ------------------------------------------------------
        def emit_gates():
            gate_cols = []
            gw_off = 0
            for gi in range(3):
                ng = NG[gi]
                x = {0: xt["sh"], 1: xt["t1"], 2: xt["t2"]}[gi]
                nd = {0: DF // 128, 1: D1 // 128, 2: D2 // 128}[gi]
                lg = ps1.tile([ng, BC], F32, name="lg", tag="p1")
                for di in range(nd):
                    nc.tensor.matmul(
                        lg[:], gwt[:, gw_off + di * ng: gw_off + (di + 1) * ng],
                        x[:, di * BC:(di + 1) * BC],
                        start=(di == 0), stop=(di == nd - 1))
                gw_off += nd * ng
                eg = gp.tile([ng, BC], BF16, name=f"eg{gi}", tag=f"eg{gi}")
                nc.scalar.activation(eg[:], lg[:],
                                     mybir.ActivationFunctionType.Exp,
                                     bias=gbt[gi * 32:gi * 32 + ng, :])
                cols = []
                for bi in range(NB):
                    pt = ps2.tile([128, ng], BF16, name="gtr", tag="p2")
                    nc.tensor.transpose(pt[:], eg[:, bi * 128:(bi + 1) * 128],
                                        ident[:ng, :ng])
                    ct = gp.tile([128, ng], F32, name=f"gc{gi}_{bi}",
                                 tag=f"gc{gi}_{bi}")
                    nc.vector.tensor_copy(ct[:], pt[:])
                    st = gp.tile([128, 1], F32, name=f"gs{gi}_{bi}",
                                 tag=f"gs{gi}_{bi}")
                    nc.vector.tensor_reduce(st[:], ct[:],
                                            axis=mybir.AxisListType.X,
                                            op=mybir.AluOpType.add)
                    rt = gp.tile([128, 1], F32, name=f"gr{gi}_{bi}",
                                 tag=f"gr{gi}_{bi}")
                    nc.vector.reciprocal(rt[:], st[:])
                    # pre-scale the gate columns: combine uses these directly
                    nc.vector.tensor_scalar_mul(ct[:], ct[:], rt[:])
                    cols.append(ct)
                gate_cols.append(cols)
            return gate_cols

        # ---- emission order --------------------------------------------
        # expert t1_0 (d-outer, earliest possible PE start), t1_1, t1_2,
        # then gates (xf has landed by then), then the deferred combines
        # and the remaining experts.
        h0 = emit_expert_l1("t1", 0, nd0, d_outer=True)
        eos_0, _ = emit_expert_l2("t1", 0, h0)
        h1 = emit_expert_l1("t1", 1, nd0)
        eos_1, _ = emit_expert_l2("t1", 1, h1)
        h2 = emit_expert_l1("t1", 2, nd0)
        eos_2, _ = emit_expert_l2("t1", 2, h2)
        gate_cols = emit_gates()
        emit_combine("t1", 0, eos_0, gate_cols)
        emit_combine("t1", 1, eos_1, gate_cols)
        emit_combine("t1", 2, eos_2, gate_cols)
        for (group, _, _, nd) in expert_groups:
            for e in range(E):
                if group == "t1" and e in (0, 1, 2):
                    continue
                h = emit_expert_l1(group, e, nd)
                eos, p2s = emit_expert_l2(group, e, h)
                emit_combine(group, e, eos, gate_cols, p2s)


_NC_CACHE = None


def _pack_xT(x_rows):
    """[BC, D] -> [128, (D/128)*BC] bf16: [p, di*BC + b] = x[b, di*128+p]."""
    bc, d = x_rows.shape
    nd = d // 128
    return np.ascontiguousarray(
        x_rows.T.reshape(nd, 128, bc).transpose(1, 0, 2).reshape(128, nd * bc)
    ).astype(NPBF16)


def _pack_inputs(inputs):
    """Host-side packing into SBUF partition layouts (pure relayout)."""
    def pack_w1(w):           # [E, D, HID] -> [E, 128, (D/128)*HID] bf16
        e, dd, nn = w.shape
        nd = dd // 128
        return np.ascontiguousarray(
            w.reshape(e, nd, 128, nn).transpose(0, 2, 1, 3).reshape(e, 128, nd * nn)
        ).astype(NPBF16)

    def pack_w2(w):           # [E, HID, OUT] -> [128, E*NH*OUT] bf16
        e, hh, nn = w.shape
        nh = hh // 128
        return np.ascontiguousarray(
            w.reshape(e, nh, 128, nn).transpose(2, 0, 1, 3).reshape(128, e * nh * nn)
        ).astype(NPBF16)

    def pack_gw(w):           # [D, ng] -> [128, (D/128)*ng]
        dd, ng = w.shape
        nd = dd // 128
        return np.ascontiguousarray(
            w.reshape(nd, 128, ng).transpose(1, 0, 2).reshape(128, nd * ng))

    # b1 blob [128, 3*E*NH] fp32: [p, (gi*E+e)*NH + hi] = b1[gi][e, hi*128+p]
    b1 = np.stack([np.asarray(inputs[f"{g}_b1"], np.float32)
                   .reshape(E, NH, 128).transpose(2, 0, 1)
                   for g in ("t1", "sh", "t2")], axis=1)   # [128, 3, E, NH]
    b1blob = np.ascontiguousarray(b1.reshape(128, 3 * E * NH), np.float32)

    # b2 blob [128, 3*E*OUT] bf16, partition-replicated rows
    b2 = np.concatenate([np.asarray(inputs[f"{g}_b2"], np.float32).reshape(E * OUT)
                         for g in ("t1", "sh", "t2")])     # [3*E*OUT]
    b2blob = np.ascontiguousarray(
        np.broadcast_to(b2[None, :], (128, 3 * E * OUT))).astype(NPBF16)

    gwblob = np.ascontiguousarray(np.concatenate(
        [pack_gw(np.asarray(inputs["gsh_W"], np.float32)),
         pack_gw(np.asarray(inputs["g1_W"], np.float32)),
         pack_gw(np.asarray(inputs["g2_W"], np.float32))], axis=1)).astype(NPBF16)
    gbblob = np.zeros((96, 1), np.float32)
    gbblob[0:12, 0] = np.asarray(inputs["gsh_b"], np.float32)
    gbblob[32:40, 0] = np.asarray(inputs["g1_b"], np.float32)
    gbblob[64:72, 0] = np.asarray(inputs["g2_b"], np.float32)

    return {
        "t1W1": pack_w1(inputs["t1_W1"]), "t2W1": pack_w1(inputs["t2_W1"]),
        "shW1": pack_w1(inputs["sh_W1"]),
        "t1W2": pack_w2(inputs["t1_W2"]), "t2W2": pack_w2(inputs["t2_W2"]),
        "shW2": pack_w2(inputs["sh_W2"]),
        "b1blob": b1blob, "b2blob": b2blob,
        "gwblob": gwblob, "gbblob": gbblob,
    }


def kernel(**inputs):
    global _NC_CACHE
    if _NC_CACHE is None:
        _NC_CACHE = _build_nc()
    nc = _NC_CACHE

    shared = _pack_inputs(inputs)
    xf, x1, x2 = inputs["x_full"], inputs["x_task1"], inputs["x_task2"]

    in_maps = []
    for c in range(N_CORES):
        rows = slice(c * BC, (c + 1) * BC)
        m = dict(shared)
        m["xfT"] = _pack_xT(xf[rows])
        m["x1T"] = _pack_xT(x1[rows])
        m["x2T"] = _pack_xT(x2[rows])
        in_maps.append(m)

    res = run_bass_kernel_spmd(nc, in_maps, list(range(N_CORES)))
    out_sh = np.concatenate([res.results[c]["out_sh"] for c in range(N_CORES)])
    out1 = np.concatenate([res.results[c]["out1"] for c in range(N_CORES)])
    out2 = np.concatenate([res.results[c]["out2"] for c in range(N_CORES)])
    return (out_sh, out1, out2)
